# revision 5
# baseline (speedup 1.0000x reference)
"""EntmaxBisect (alpha=1.5, N_ITER=50, dim=-1) Trainium2 Bass kernel.

Math (host-validated, 0 active-set mismatches vs the f32 jax reference on
the seed-0 input): with p = 1/4095, u^p in [0.9958, 1.0002] for every
positive f32 u, so the normalized output is 1/k on the k elements at or
above the bisection threshold and 0 elsewhere (max elementwise deviation
2.3e-3, aggregate 1.6e-3 vs the 2e-2 gate).  The 50-step bisection
threshold collapses to twot = clamp(s2, m-2, m-1/32) in x units, where
s2 = max over non-max elements; duplicated max (cnt >= 2) converges to
t_max, i.e. twot = m - 1/32 exactly.

Engine-legal pass structure (BIR-verifier constraints: Pool = elementwise
tensor_scalar / tensor_tensor only, no accum, no stt; accums live on DVE
and ACT; hw-probed Sign(0) == 0):
  P1   m    = max-accum(x)                 tensor_scalar      [DVE]
  L    L    = Sign(m - x)  (1 below max, 0 at copies)
       cnt  = 4096 - add-accum(L)          activation         [ACT]
  PEN  pen  = L * x   (in-place into L; 0 at copies, s2 > 0)
                                           tensor_tensor      [Pool]
  S2   s2   = max-accum(pen)  (max select is exact)           [DVE]
  P45  mask = (x >= twot), k = add-accum  (in-place, full)    [DVE]
  R    r    = reciprocal(k)                                   [DVE tiny]
  P5   out  = mask * r  (in-place, column-split)       [DVE+Pool+ACT]
Tiny twot chain on Pool (ts-with-column-scalar forms only; dup override
arithmetic: tw = max(tw, dup*1e30 - 1e30 + (m-1/32))).
DMA: loads SP/ACT, stores SP/ACT/Pool (independent queues in the cost
model; DVE cannot DMA).  Software pipeline lags: L/pen@t, s2@t-1,
tiny@t-1, p45@t-2, r@t-3, p5/stores@t-4, over an NSLOT=8 slot ring.
Sharding: batch dim across the 8 cores, tile = 128 rows x 4096.
"""
import numpy as np
import concourse.bass as bass
import concourse.mybir as mybir
from concourse.bass_utils import run_bass_kernel_spmd
from contextlib import ExitStack

f32 = mybir.dt.float32
Alu = mybir.AluOpType
Act = mybir.ActivationFunctionType

B, S, D = 8, 2048, 4096
NCORES = 8
R = B * S // NCORES            # rows per core (2048)
PT = 128                       # partitions per tile
NT = R // PT                   # 16 tiles per core
LT = NT - 1                    # last tile: P5 fully on DVE at the tail

NSLOT = 8
LSP = 3328                     # load cols on SP queue; rest on ACT
SPL = 1408                     # store cols on Pool queue
SSP = 1920                     # store cols on SP queue (after Pool's)
# ACT stores the remainder [SPL+SSP : D) == its own load range; the
# queue-decoupling requires SPL + SSP == LSP (loads on one queue must
# not overlap stores pending on another queue's semaphore)
P5D = 1792                     # P5 cols on DVE
P5P = 1152                     # P5 cols on Pool; ACT takes the rest

_cached = {}


def _build(**over):
    g = dict(NSLOT=NSLOT, LSP=LSP, SPL=SPL, SSP=SSP, P5D=P5D, P5P=P5P)
    g.update(over)
    NSLOT_, LSP_, SPL_, SSP_, P5D_, P5P_ = (g["NSLOT"], g["LSP"], g["SPL"],
                                            g["SSP"], g["P5D"], g["P5P"])
    nc = bass.Bass(detect_race_conditions=False)
    x_in = nc.dram_tensor("x", [R, D], f32, kind="ExternalInput")
    out_dr = nc.dram_tensor("out", [R, D], f32, kind="ExternalOutput")

    with ExitStack() as st:
        block = st.enter_context(nc.Block())

        def sem(name):
            return st.enter_context(nc.semaphore(name))

        dLsp, dLact, dLpool = sem("dLsp"), sem("dLact"), sem("dLpool")
        dSsp, dSact, dSpool = sem("dSsp"), sem("dSact"), sem("dSpool")
        sM, sL, sPen, sS2 = sem("sM"), sem("sL"), sem("sPen"), sem("sS2")
        sTw, sR = sem("sTw"), sem("sR")
        sP5d, sP5p, sP5a, sP5L = (sem("sP5d"), sem("sP5p"), sem("sP5a"),
                                  sem("sP5L"))

        def sb(name, shape, dt=f32):
            return st.enter_context(nc.sbuf_tensor(name, shape, dt))

        xsl = [sb(f"x{i}", [PT, D]) for i in range(NSLOT_)]
        Lb = [sb(f"L{i}", [PT, D]) for i in range(3)]
        junk = sb("junk", [PT, D])
        m = sb("m", [PT, NT])
        sg = sb("sg", [PT, NT])
        s2 = sb("s2", [PT, NT])
        ta = sb("ta", [PT, NT])
        tb = sb("tb", [PT, NT])
        tw = sb("tw", [PT, NT])
        qq = sb("qq", [PT, NT])
        kc = sb("kc", [PT, NT])
        rc = sb("rc", [PT, NT])

        def c(t):
            return slice(t, t + 1)

        P5A0 = P5D_ + P5P_      # ACT P5 range start

        @block.sync
        def _(sync):
            for t in range(4):
                sync.dma_start(
                    xsl[t][:, 0:1536], x_in[t * PT:(t + 1) * PT, 0:1536]
                ).then_inc(dLsp, 16)
            for t in range(4, NSLOT_):
                sync.dma_start(
                    xsl[t][:, 0:LSP_], x_in[t * PT:(t + 1) * PT, 0:LSP_]
                ).then_inc(dLsp, 16)
            for t in range(NT):
                sync.wait_ge(sP5d, t + 1)
                sync.wait_ge(sP5p, t + 1)
                sync.wait_ge(sP5a, t + 1)
                c0, c1 = (SPL_, SPL_ + SSP_) if t < 10 else (1365, 2731)
                sync.dma_start(
                    out_dr[t * PT:(t + 1) * PT, c0:c1],
                    xsl[t % NSLOT_][:, c0:c1]
                ).then_inc(dSsp, 16)
                tn = t + NSLOT_
                if tn < NT:
                    sync.wait_ge(dSpool, 16 * (t + 1))
                    sync.dma_start(
                        xsl[tn % NSLOT_][:, 0:LSP_],
                        x_in[tn * PT:(tn + 1) * PT, 0:LSP_]
                    ).then_inc(dLsp, 16)
            sync.wait_ge(dSact, 16 * NT)
            sync.wait_ge(dSpool, 16 * NT)
            sync.wait_ge(dSsp, 16 * NT)

        @block.vector
        def _(v):
            def p1(t):
                v.wait_ge(dLsp, 16 * (t + 1))
                v.wait_ge(dLact, 16 * (t + 1))
                if t < 4:
                    v.wait_ge(dLpool, 16 * (t + 1))
                v.tensor_scalar(junk[:], xsl[t % NSLOT_][:], 0.0, None,
                                op0=Alu.bypass, op1=Alu.max,
                                accum_out=m[:, c(t)]).then_inc(sM, 1)

            def s2op(t):
                v.wait_ge(sPen, t + 1)
                v.tensor_scalar(junk[:], Lb[t % 3][:], 0.0, None,
                                op0=Alu.bypass, op1=Alu.max,
                                accum_out=s2[:, c(t)]).then_inc(sS2, 1)

            def p45(t):
                v.wait_ge(sTw, t + 1)
                v.tensor_scalar(xsl[t % NSLOT_][:], xsl[t % NSLOT_][:],
                                tw[:, c(t)], None,
                                op0=Alu.is_ge, op1=Alu.add,
                                accum_out=kc[:, c(t)])

            def recip(t):
                v.reciprocal(rc[:, c(t)], kc[:, c(t)]).then_inc(sR, 1)

            def p5d(t):
                v.tensor_scalar(xsl[t % NSLOT_][:, 0:P5D_],
                                xsl[t % NSLOT_][:, 0:P5D_], rc[:, c(t)], None,
                                op0=Alu.mult).then_inc(sP5d, 1)

            for t in range(NT):
                p1(t)
                if t >= 6:
                    p5d(t - 6)
                if t >= 5:
                    recip(t - 5)
                if t >= 2:
                    s2op(t - 2)
                if t >= 4:
                    p45(t - 4)
            s2op(NT - 2)
            p45(NT - 4)
            recip(NT - 5)
            p5d(NT - 6)
            s2op(NT - 1)
            p45(NT - 3)
            recip(NT - 4)
            p5d(NT - 5)
            p45(NT - 2)
            recip(NT - 3)
            p5d(NT - 4)
            p45(NT - 1)
            recip(NT - 2)
            p5d(NT - 3)
            recip(NT - 1)
            p5d(NT - 2)
            p5d(NT - 1)

        @block.scalar
        def _(s):
            def sign_op(t):
                s.wait_ge(sM, t + 1)
                if t >= 3:
                    s.wait_ge(sS2, t - 2)    # Lb[t%3] free again
                s.activation(Lb[t % 3][:], xsl[t % NSLOT_][:], Act.Sign,
                             bias=m[:, c(t)], scale=-1.0,
                             accum_out=sg[:, c(t)]).then_inc(sL, 1)

            def p5a(t):
                s.wait_ge(sR, t + 1)
                s.activation(xsl[t % NSLOT_][:, P5A0:D],
                             xsl[t % NSLOT_][:, P5A0:D],
                             Act.Copy, bias=0.0,
                             scale=rc[:, c(t)]).then_inc(sP5a, 1)

            def store_act(t):
                s.wait_ge(sP5d, t + 1)
                s.wait_ge(sP5p, t + 1)
                c0 = SPL_ + SSP_ if t < 10 else 2731
                s.dma_start(
                    out_dr[t * PT:(t + 1) * PT, c0:D],
                    xsl[t % NSLOT_][:, c0:D]
                ).then_inc(dSact, 16)

            for t in range(2):
                s.dma_start(
                    xsl[t][:, 1536:2816], x_in[t * PT:(t + 1) * PT, 1536:2816]
                ).then_inc(dLact, 16)
            for t in range(NT):
                if t < 2:
                    s.dma_start(
                        xsl[t + 2][:, 1536:2816],
                        x_in[(t + 2) * PT:(t + 3) * PT, 1536:2816]
                    ).then_inc(dLact, 16)
                if t + 4 < NSLOT_:
                    s.dma_start(
                        xsl[t + 4][:, LSP_:D],
                        x_in[(t + 4) * PT:(t + 5) * PT, LSP_:D]
                    ).then_inc(dLact, 16)
                if t >= 6:
                    p5a(t - 6)
                    store_act(t - 6)
                    tn = t - 6 + NSLOT_
                    if tn < NT:
                        s.dma_start(
                            xsl[tn % NSLOT_][:, LSP_:D],
                            x_in[tn * PT:(tn + 1) * PT, LSP_:D]
                        ).then_inc(dLact, 16)
                sign_op(t)
            for t in range(NT - 6, NT):
                p5a(t)
                store_act(t)
            s.wait_ge(dSsp, 16 * NT)

        @block.gpsimd
        def _(gp):
            def pen_op(t):
                gp.wait_ge(sL, t + 1)
                gp.tensor_tensor(out=Lb[t % 3][:], in0=Lb[t % 3][:],
                                 in1=xsl[t % NSLOT_][:],
                                 op=Alu.mult).then_inc(sPen, 1)

            def tiny_op(t):
                gp.tensor_scalar(ta[:, c(t)], m[:, c(t)], 2.0, None,
                                 op0=Alu.subtract)
                gp.tensor_scalar(tb[:, c(t)], m[:, c(t)], 0.03125, None,
                                 op0=Alu.subtract)
                # dup <=> cnt >= 2 <=> sum(L) <= 4094
                gp.tensor_scalar(qq[:, c(t)], sg[:, c(t)], float(D - 2) + 0.5,
                                 None, op0=Alu.is_le)
                gp.drain()
                gp.tensor_scalar(qq[:, c(t)], qq[:, c(t)], 1e30, 1e30,
                                 op0=Alu.mult, op1=Alu.subtract)
                gp.drain()
                gp.tensor_scalar(qq[:, c(t)], qq[:, c(t)], tb[:, c(t)], None,
                                 op0=Alu.add)
                gp.wait_ge(sS2, t + 1)
                gp.tensor_scalar(tw[:, c(t)], s2[:, c(t)], ta[:, c(t)], None,
                                 op0=Alu.max)
                gp.drain()
                gp.tensor_scalar(tw[:, c(t)], tw[:, c(t)], tb[:, c(t)], None,
                                 op0=Alu.min)
                gp.drain()
                gp.tensor_scalar(tw[:, c(t)], tw[:, c(t)], qq[:, c(t)], None,
                                 op0=Alu.max)
                gp.drain().then_inc(sTw, 1)

            def p5p(t):
                gp.wait_ge(sR, t + 1)
                gp.tensor_scalar(xsl[t % NSLOT_][:, P5D_:P5A0],
                                 xsl[t % NSLOT_][:, P5D_:P5A0],
                                 rc[:, c(t)], None,
                                 op0=Alu.mult).then_inc(sP5p, 1)

            def store_pool(t):
                gp.wait_ge(sP5d, t + 1)
                gp.wait_ge(sP5a, t + 1)
                gp.wait_ge(sP5p, t + 1)
                c1 = SPL_ if t < 10 else 1365
                gp.dma_start(
                    out_dr[t * PT:(t + 1) * PT, 0:c1],
                    xsl[t % NSLOT_][:, 0:c1]
                ).then_inc(dSpool, 16)

            for t in range(4):
                gp.dma_start(
                    xsl[t][:, 2816:D], x_in[t * PT:(t + 1) * PT, 2816:D]
                ).then_inc(dLpool, 16)
            for t in range(NT):
                if t >= 6:
                    p5p(t - 6)
                    store_pool(t - 6)
                if t >= 1:
                    pen_op(t - 1)
                if t >= 3:
                    tiny_op(t - 3)
            pen_op(NT - 1)
            tiny_op(NT - 3)
            tiny_op(NT - 2)
            tiny_op(NT - 1)
            for t in range(NT - 6, NT):
                p5p(t)
                store_pool(t)

    return nc


def kernel(X: np.ndarray) -> np.ndarray:
    assert X.shape == (B, S, D) and X.dtype == np.float32
    if "nc" not in _cached:
        _cached["nc"] = _build()
    nc = _cached["nc"]
    in_maps = [{"x": np.ascontiguousarray(X[c])} for c in range(NCORES)]
    res = run_bass_kernel_spmd(nc, in_maps, core_ids=list(range(NCORES)))
    out = np.stack([res.results[c]["out"] for c in range(NCORES)], axis=0)
    return out


# revision 6
# speedup vs baseline: 1.0077x; 1.0077x over previous
"""EntmaxBisect (alpha=1.5, N_ITER=50, dim=-1) Trainium2 Bass kernel.

Math (host-validated, 0 active-set mismatches vs the f32 jax reference on
the seed-0 input): with p = 1/4095, u^p in [0.9958, 1.0002] for every
positive f32 u, so the normalized output is 1/k on the k elements at or
above the bisection threshold and 0 elsewhere (max elementwise deviation
2.3e-3, aggregate 1.6e-3 vs the 2e-2 gate).  The 50-step bisection
threshold collapses to twot = clamp(s2, m-2, m-1/32) in x units, where
s2 = max over non-max elements; duplicated max (cnt >= 2) converges to
t_max, i.e. twot = m - 1/32 exactly.

Engine-legal pass structure (BIR-verifier constraints: Pool = elementwise
tensor_scalar / tensor_tensor only, no accum, no stt; accums live on DVE
and ACT; hw-probed Sign(0) == 0):
  P1   m    = max-accum(x)                 tensor_scalar      [DVE]
  L    L    = Sign(m - x)  (1 below max, 0 at copies)
       cnt  = 4096 - add-accum(L)          activation         [ACT]
  PEN  pen  = L * x   (in-place into L; 0 at copies, s2 > 0)
                                           tensor_tensor      [Pool]
  S2   s2   = max-accum(pen)  (max select is exact)           [DVE]
  P45  mask = (x >= twot), k = add-accum  (in-place, full)    [DVE]
  R    r    = reciprocal(k)                                   [DVE tiny]
  P5   out  = mask * r  (in-place, column-split)       [DVE+Pool+ACT]
Tiny twot chain on Pool (ts-with-column-scalar forms only; dup override
arithmetic: tw = max(tw, dup*1e30 - 1e30 + (m-1/32))).
DMA: loads SP/ACT, stores SP/ACT/Pool (independent queues in the cost
model; DVE cannot DMA).  Software pipeline lags: L/pen@t, s2@t-1,
tiny@t-1, p45@t-2, r@t-3, p5/stores@t-4, over an NSLOT=8 slot ring.
Sharding: batch dim across the 8 cores, tile = 128 rows x 4096.
"""
import numpy as np
import concourse.bass as bass
import concourse.mybir as mybir
from concourse.bass_utils import run_bass_kernel_spmd
from contextlib import ExitStack

f32 = mybir.dt.float32
Alu = mybir.AluOpType
Act = mybir.ActivationFunctionType

B, S, D = 8, 2048, 4096
NCORES = 8
R = B * S // NCORES            # rows per core (2048)
PT = 128                       # partitions per tile
NT = R // PT                   # 16 tiles per core
LT = NT - 1                    # last tile: P5 fully on DVE at the tail

NSLOT = 8
LSP = 3328                     # load cols on SP queue; rest on ACT
SPL = 1408                     # store cols on Pool queue
SSP = 1920                     # store cols on SP queue (after Pool's)
# ACT stores the remainder [SPL+SSP : D) == its own load range; the
# queue-decoupling requires SPL + SSP == LSP (loads on one queue must
# not overlap stores pending on another queue's semaphore)
P5D = 1792                     # P5 cols on DVE
P5P = 1152                     # P5 cols on Pool; ACT takes the rest

_cached = {}


def _build(**over):
    g = dict(NSLOT=NSLOT, LSP=LSP, SPL=SPL, SSP=SSP, P5D=P5D, P5P=P5P)
    g.update(over)
    NSLOT_, LSP_, SPL_, SSP_, P5D_, P5P_ = (g["NSLOT"], g["LSP"], g["SPL"],
                                            g["SSP"], g["P5D"], g["P5P"])
    nc = bass.Bass(detect_race_conditions=False)
    x_in = nc.dram_tensor("x", [R, D], f32, kind="ExternalInput")
    out_dr = nc.dram_tensor("out", [R, D], f32, kind="ExternalOutput")

    with ExitStack() as st:
        block = st.enter_context(nc.Block())

        def sem(name):
            return st.enter_context(nc.semaphore(name))

        dLsp, dLact, dLpool = sem("dLsp"), sem("dLact"), sem("dLpool")
        dSsp, dSact, dSpool = sem("dSsp"), sem("dSact"), sem("dSpool")
        sM, sL, sPen, sS2 = sem("sM"), sem("sL"), sem("sPen"), sem("sS2")
        sTw, sR = sem("sTw"), sem("sR")
        sP5d, sP5p, sP5a, sP5L = (sem("sP5d"), sem("sP5p"), sem("sP5a"),
                                  sem("sP5L"))

        def sb(name, shape, dt=f32):
            return st.enter_context(nc.sbuf_tensor(name, shape, dt))

        xsl = [sb(f"x{i}", [PT, D]) for i in range(NSLOT_)]
        Lb = [sb(f"L{i}", [PT, D]) for i in range(3)]
        junk = sb("junk", [PT, D])
        m = sb("m", [PT, NT])
        sg = sb("sg", [PT, NT])
        s2 = sb("s2", [PT, NT])
        ta = sb("ta", [PT, NT])
        tb = sb("tb", [PT, NT])
        tw = sb("tw", [PT, NT])
        qq = sb("qq", [PT, NT])
        kc = sb("kc", [PT, NT])
        rc = sb("rc", [PT, NT])

        def c(t):
            return slice(t, t + 1)

        P5A0 = P5D_ + P5P_      # ACT P5 range start

        @block.sync
        def _(sync):
            for t in range(4):
                sync.dma_start(
                    xsl[t][:, 0:1792], x_in[t * PT:(t + 1) * PT, 0:1792]
                ).then_inc(dLsp, 16)
            for t in range(4, NSLOT_):
                sync.dma_start(
                    xsl[t][:, 0:LSP_], x_in[t * PT:(t + 1) * PT, 0:LSP_]
                ).then_inc(dLsp, 16)
            for t in range(NT):
                sync.wait_ge(sP5d, t + 1)
                sync.wait_ge(sP5p, t + 1)
                sync.wait_ge(sP5a, t + 1)
                c0, c1 = (SPL_, SPL_ + SSP_) if t < 10 else (1365, 2731)
                sync.dma_start(
                    out_dr[t * PT:(t + 1) * PT, c0:c1],
                    xsl[t % NSLOT_][:, c0:c1]
                ).then_inc(dSsp, 16)
                tn = t + NSLOT_
                if tn < NT:
                    sync.wait_ge(dSpool, 16 * (t + 1))
                    sync.dma_start(
                        xsl[tn % NSLOT_][:, 0:LSP_],
                        x_in[tn * PT:(tn + 1) * PT, 0:LSP_]
                    ).then_inc(dLsp, 16)
            sync.wait_ge(dSact, 16 * NT)
            sync.wait_ge(dSpool, 16 * NT)
            sync.wait_ge(dSsp, 16 * NT)

        @block.vector
        def _(v):
            def p1(t):
                v.wait_ge(dLsp, 16 * (t + 1))
                v.wait_ge(dLact, 16 * (t + 1))
                if t < 4:
                    v.wait_ge(dLpool, 16 * (t + 1))
                v.tensor_scalar(junk[:], xsl[t % NSLOT_][:], 0.0, None,
                                op0=Alu.bypass, op1=Alu.max,
                                accum_out=m[:, c(t)]).then_inc(sM, 1)

            def s2op(t):
                v.wait_ge(sPen, t + 1)
                v.tensor_scalar(junk[:], Lb[t % 3][:], 0.0, None,
                                op0=Alu.bypass, op1=Alu.max,
                                accum_out=s2[:, c(t)]).then_inc(sS2, 1)

            def p45(t):
                v.wait_ge(sTw, t + 1)
                v.tensor_scalar(xsl[t % NSLOT_][:], xsl[t % NSLOT_][:],
                                tw[:, c(t)], None,
                                op0=Alu.is_ge, op1=Alu.add,
                                accum_out=kc[:, c(t)])

            def recip(t):
                v.reciprocal(rc[:, c(t)], kc[:, c(t)]).then_inc(sR, 1)

            def p5d(t):
                v.tensor_scalar(xsl[t % NSLOT_][:, 0:P5D_],
                                xsl[t % NSLOT_][:, 0:P5D_], rc[:, c(t)], None,
                                op0=Alu.mult).then_inc(sP5d, 1)

            for t in range(NT):
                p1(t)
                if t >= 6:
                    p5d(t - 6)
                if t >= 5:
                    recip(t - 5)
                if t >= 2:
                    s2op(t - 2)
                if t >= 4:
                    p45(t - 4)
            s2op(NT - 2)
            p45(NT - 4)
            recip(NT - 5)
            p5d(NT - 6)
            s2op(NT - 1)
            p45(NT - 3)
            recip(NT - 4)
            p5d(NT - 5)
            p45(NT - 2)
            recip(NT - 3)
            p5d(NT - 4)
            p45(NT - 1)
            recip(NT - 2)
            p5d(NT - 3)
            recip(NT - 1)
            p5d(NT - 2)
            p5d(NT - 1)

        @block.scalar
        def _(s):
            def sign_op(t):
                s.wait_ge(sM, t + 1)
                if t >= 3:
                    s.wait_ge(sS2, t - 2)    # Lb[t%3] free again
                s.activation(Lb[t % 3][:], xsl[t % NSLOT_][:], Act.Sign,
                             bias=m[:, c(t)], scale=-1.0,
                             accum_out=sg[:, c(t)]).then_inc(sL, 1)

            def p5a(t):
                s.wait_ge(sR, t + 1)
                s.activation(xsl[t % NSLOT_][:, P5A0:D],
                             xsl[t % NSLOT_][:, P5A0:D],
                             Act.Copy, bias=0.0,
                             scale=rc[:, c(t)]).then_inc(sP5a, 1)

            def store_act(t):
                s.wait_ge(sP5d, t + 1)
                s.wait_ge(sP5p, t + 1)
                c0 = SPL_ + SSP_ if t < 10 else 2731
                s.dma_start(
                    out_dr[t * PT:(t + 1) * PT, c0:D],
                    xsl[t % NSLOT_][:, c0:D]
                ).then_inc(dSact, 16)

            for t in range(2):
                s.dma_start(
                    xsl[t][:, 1792:2560], x_in[t * PT:(t + 1) * PT, 1792:2560]
                ).then_inc(dLact, 16)
            for t in range(NT):
                if t < 2:
                    s.dma_start(
                        xsl[t + 2][:, 1792:2560],
                        x_in[(t + 2) * PT:(t + 3) * PT, 1792:2560]
                    ).then_inc(dLact, 16)
                if t + 4 < NSLOT_:
                    s.dma_start(
                        xsl[t + 4][:, LSP_:D],
                        x_in[(t + 4) * PT:(t + 5) * PT, LSP_:D]
                    ).then_inc(dLact, 16)
                if t >= 6:
                    p5a(t - 6)
                    store_act(t - 6)
                    tn = t - 6 + NSLOT_
                    if tn < NT:
                        s.dma_start(
                            xsl[tn % NSLOT_][:, LSP_:D],
                            x_in[tn * PT:(tn + 1) * PT, LSP_:D]
                        ).then_inc(dLact, 16)
                sign_op(t)
            for t in range(NT - 6, NT):
                p5a(t)
                store_act(t)
            s.wait_ge(dSsp, 16 * NT)

        @block.gpsimd
        def _(gp):
            def pen_op(t):
                gp.wait_ge(sL, t + 1)
                gp.tensor_tensor(out=Lb[t % 3][:], in0=Lb[t % 3][:],
                                 in1=xsl[t % NSLOT_][:],
                                 op=Alu.mult).then_inc(sPen, 1)

            def tiny_op(t):
                gp.tensor_scalar(ta[:, c(t)], m[:, c(t)], 2.0, None,
                                 op0=Alu.subtract)
                gp.tensor_scalar(tb[:, c(t)], m[:, c(t)], 0.03125, None,
                                 op0=Alu.subtract)
                # dup <=> cnt >= 2 <=> sum(L) <= 4094
                gp.tensor_scalar(qq[:, c(t)], sg[:, c(t)], float(D - 2) + 0.5,
                                 None, op0=Alu.is_le)
                gp.drain()
                gp.tensor_scalar(qq[:, c(t)], qq[:, c(t)], 1e30, 1e30,
                                 op0=Alu.mult, op1=Alu.subtract)
                gp.drain()
                gp.tensor_scalar(qq[:, c(t)], qq[:, c(t)], tb[:, c(t)], None,
                                 op0=Alu.add)
                gp.wait_ge(sS2, t + 1)
                gp.tensor_scalar(tw[:, c(t)], s2[:, c(t)], ta[:, c(t)], None,
                                 op0=Alu.max)
                gp.drain()
                gp.tensor_scalar(tw[:, c(t)], tw[:, c(t)], tb[:, c(t)], None,
                                 op0=Alu.min)
                gp.drain()
                gp.tensor_scalar(tw[:, c(t)], tw[:, c(t)], qq[:, c(t)], None,
                                 op0=Alu.max)
                gp.drain().then_inc(sTw, 1)

            def p5p(t):
                gp.wait_ge(sR, t + 1)
                gp.tensor_scalar(xsl[t % NSLOT_][:, P5D_:P5A0],
                                 xsl[t % NSLOT_][:, P5D_:P5A0],
                                 rc[:, c(t)], None,
                                 op0=Alu.mult).then_inc(sP5p, 1)

            def store_pool(t):
                gp.wait_ge(sP5d, t + 1)
                gp.wait_ge(sP5a, t + 1)
                gp.wait_ge(sP5p, t + 1)
                c1 = SPL_ if t < 10 else 1365
                gp.dma_start(
                    out_dr[t * PT:(t + 1) * PT, 0:c1],
                    xsl[t % NSLOT_][:, 0:c1]
                ).then_inc(dSpool, 16)

            for t in range(4):
                gp.dma_start(
                    xsl[t][:, 2560:D], x_in[t * PT:(t + 1) * PT, 2560:D]
                ).then_inc(dLpool, 16)
            for t in range(NT):
                if t >= 6:
                    p5p(t - 6)
                    store_pool(t - 6)
                if t >= 1:
                    pen_op(t - 1)
                if t >= 3:
                    tiny_op(t - 3)
            pen_op(NT - 1)
            tiny_op(NT - 3)
            tiny_op(NT - 2)
            tiny_op(NT - 1)
            for t in range(NT - 6, NT):
                p5p(t)
                store_pool(t)

    return nc


def kernel(X: np.ndarray) -> np.ndarray:
    assert X.shape == (B, S, D) and X.dtype == np.float32
    if "nc" not in _cached:
        _cached["nc"] = _build()
    nc = _cached["nc"]
    in_maps = [{"x": np.ascontiguousarray(X[c])} for c in range(NCORES)]
    res = run_bass_kernel_spmd(nc, in_maps, core_ids=list(range(NCORES)))
    out = np.stack([res.results[c]["out"] for c in range(NCORES)], axis=0)
    return out


# revision 7
# speedup vs baseline: 1.0104x; 1.0027x over previous
"""EntmaxBisect (alpha=1.5, N_ITER=50, dim=-1) Trainium2 Bass kernel.

Math (host-validated, 0 active-set mismatches vs the f32 jax reference on
the seed-0 input): with p = 1/4095, u^p in [0.9958, 1.0002] for every
positive f32 u, so the normalized output is 1/k on the k elements at or
above the bisection threshold and 0 elsewhere (max elementwise deviation
2.3e-3, aggregate 1.6e-3 vs the 2e-2 gate).  The 50-step bisection
threshold collapses to twot = clamp(s2, m-2, m-1/32) in x units, where
s2 = max over non-max elements; duplicated max (cnt >= 2) converges to
t_max, i.e. twot = m - 1/32 exactly.

Engine-legal pass structure (BIR-verifier constraints: Pool = elementwise
tensor_scalar / tensor_tensor only, no accum, no stt; accums live on DVE
and ACT; hw-probed Sign(0) == 0):
  P1   m    = max-accum(x)                 tensor_scalar      [DVE]
  L    L    = Sign(m - x)  (1 below max, 0 at copies)
       cnt  = 4096 - add-accum(L)          activation         [ACT]
  PEN  pen  = L * x   (in-place into L; 0 at copies, s2 > 0)
                                           tensor_tensor      [Pool]
  S2   s2   = max-accum(pen)  (max select is exact)           [DVE]
  P45  mask = (x >= twot), k = add-accum  (in-place, full)    [DVE]
  R    r    = reciprocal(k)                                   [DVE tiny]
  P5   out  = mask * r  (in-place, column-split)       [DVE+Pool+ACT]
Tiny twot chain on Pool (ts-with-column-scalar forms only; dup override
arithmetic: tw = max(tw, dup*1e30 - 1e30 + (m-1/32))).
DMA: loads SP/ACT, stores SP/ACT/Pool (independent queues in the cost
model; DVE cannot DMA).  Software pipeline lags: L/pen@t, s2@t-1,
tiny@t-1, p45@t-2, r@t-3, p5/stores@t-4, over an NSLOT=8 slot ring.
Sharding: batch dim across the 8 cores, tile = 128 rows x 4096.
"""
import numpy as np
import concourse.bass as bass
import concourse.mybir as mybir
from concourse.bass_utils import run_bass_kernel_spmd
from contextlib import ExitStack

f32 = mybir.dt.float32
Alu = mybir.AluOpType
Act = mybir.ActivationFunctionType

B, S, D = 8, 2048, 4096
NCORES = 8
R = B * S // NCORES            # rows per core (2048)
PT = 128                       # partitions per tile
NT = R // PT                   # 16 tiles per core
LT = NT - 1                    # last tile: P5 fully on DVE at the tail

NSLOT = 8
LSP = 3328                     # load cols on SP queue; rest on ACT
SPL = 1408                     # store cols on Pool queue
SSP = 1920                     # store cols on SP queue (after Pool's)
# ACT stores the remainder [SPL+SSP : D) == its own load range; the
# queue-decoupling requires SPL + SSP == LSP (loads on one queue must
# not overlap stores pending on another queue's semaphore)
P5D = 1792                     # P5 cols on DVE
P5P = 1152                     # P5 cols on Pool; ACT takes the rest

_cached = {}


def _build(**over):
    g = dict(NSLOT=NSLOT, LSP=LSP, SPL=SPL, SSP=SSP, P5D=P5D, P5P=P5P)
    g.update(over)
    NSLOT_, LSP_, SPL_, SSP_, P5D_, P5P_ = (g["NSLOT"], g["LSP"], g["SPL"],
                                            g["SSP"], g["P5D"], g["P5P"])
    nc = bass.Bass(detect_race_conditions=False)
    x_in = nc.dram_tensor("x", [R, D], f32, kind="ExternalInput")
    out_dr = nc.dram_tensor("out", [R, D], f32, kind="ExternalOutput")

    with ExitStack() as st:
        block = st.enter_context(nc.Block())

        def sem(name):
            return st.enter_context(nc.semaphore(name))

        dLsp, dLact, dLpool = sem("dLsp"), sem("dLact"), sem("dLpool")
        dLp2 = sem("dLp2")
        dSsp, dSact, dSpool = sem("dSsp"), sem("dSact"), sem("dSpool")
        sM, sL, sPen, sS2 = sem("sM"), sem("sL"), sem("sPen"), sem("sS2")
        sTw, sR = sem("sTw"), sem("sR")
        sP5d, sP5p, sP5a, sP5L = (sem("sP5d"), sem("sP5p"), sem("sP5a"),
                                  sem("sP5L"))

        def sb(name, shape, dt=f32):
            return st.enter_context(nc.sbuf_tensor(name, shape, dt))

        xsl = [sb(f"x{i}", [PT, D]) for i in range(NSLOT_)]
        Lb = [sb(f"L{i}", [PT, D]) for i in range(3)]
        junk = sb("junk", [PT, D])
        m = sb("m", [PT, NT])
        sg = sb("sg", [PT, NT])
        s2 = sb("s2", [PT, NT])
        ta = sb("ta", [PT, NT])
        tb = sb("tb", [PT, NT])
        tw = sb("tw", [PT, NT])
        qq = sb("qq", [PT, NT])
        kc = sb("kc", [PT, NT])
        rc = sb("rc", [PT, NT])

        def c(t):
            return slice(t, t + 1)

        P5A0 = P5D_ + P5P_      # ACT P5 range start

        @block.sync
        def _(sync):
            for t in range(4):
                sync.dma_start(
                    xsl[t][:, 0:1792], x_in[t * PT:(t + 1) * PT, 0:1792]
                ).then_inc(dLsp, 16)
            for t in range(4, NSLOT_):
                sync.dma_start(
                    xsl[t][:, 0:LSP_], x_in[t * PT:(t + 1) * PT, 0:LSP_]
                ).then_inc(dLsp, 16)
            for t in range(NT):
                sync.wait_ge(sP5d, t + 1)
                sync.wait_ge(sP5p, t + 1)
                sync.wait_ge(sP5a, t + 1)
                c0, c1 = (SPL_, SPL_ + SSP_) if t < 10 else (1365, 2731)
                sync.dma_start(
                    out_dr[t * PT:(t + 1) * PT, c0:c1],
                    xsl[t % NSLOT_][:, c0:c1]
                ).then_inc(dSsp, 16)
                tn = t + NSLOT_
                if tn < NT:
                    sync.wait_ge(dSpool, 16 * (t + 1))
                    sync.dma_start(
                        xsl[tn % NSLOT_][:, 0:LSP_],
                        x_in[tn * PT:(tn + 1) * PT, 0:LSP_]
                    ).then_inc(dLsp, 16)
            sync.wait_ge(dSact, 16 * NT)
            sync.wait_ge(dSpool, 16 * NT)
            sync.wait_ge(dSsp, 16 * NT)

        @block.vector
        def _(v):
            def p1(t):
                v.wait_ge(dLsp, 16 * (t + 1))
                if t < 4:
                    v.wait_ge(dLact, 16 * (t + 1))
                    v.wait_ge(dLpool, 16 * (t + 1))
                elif t < 8:
                    v.wait_ge(dLp2, 16 * (t - 3))
                else:
                    v.wait_ge(dLact, 16 * (t - 3))
                v.tensor_scalar(junk[:], xsl[t % NSLOT_][:], 0.0, None,
                                op0=Alu.bypass, op1=Alu.max,
                                accum_out=m[:, c(t)]).then_inc(sM, 1)

            def s2op(t):
                v.wait_ge(sPen, t + 1)
                v.tensor_scalar(junk[:], Lb[t % 3][:], 0.0, None,
                                op0=Alu.bypass, op1=Alu.max,
                                accum_out=s2[:, c(t)]).then_inc(sS2, 1)

            def p45(t):
                v.wait_ge(sTw, t + 1)
                v.tensor_scalar(xsl[t % NSLOT_][:], xsl[t % NSLOT_][:],
                                tw[:, c(t)], None,
                                op0=Alu.is_ge, op1=Alu.add,
                                accum_out=kc[:, c(t)])

            def recip(t):
                v.reciprocal(rc[:, c(t)], kc[:, c(t)]).then_inc(sR, 1)

            def p5d(t):
                v.tensor_scalar(xsl[t % NSLOT_][:, 0:P5D_],
                                xsl[t % NSLOT_][:, 0:P5D_], rc[:, c(t)], None,
                                op0=Alu.mult).then_inc(sP5d, 1)

            for t in range(NT):
                p1(t)
                if t >= 6:
                    p5d(t - 6)
                if t >= 5:
                    recip(t - 5)
                if t >= 2:
                    s2op(t - 2)
                if t >= 4:
                    p45(t - 4)
            s2op(NT - 2)
            p45(NT - 4)
            recip(NT - 5)
            p5d(NT - 6)
            s2op(NT - 1)
            p45(NT - 3)
            recip(NT - 4)
            p5d(NT - 5)
            p45(NT - 2)
            recip(NT - 3)
            p5d(NT - 4)
            p45(NT - 1)
            recip(NT - 2)
            p5d(NT - 3)
            recip(NT - 1)
            p5d(NT - 2)
            p5d(NT - 1)

        @block.scalar
        def _(s):
            def sign_op(t):
                s.wait_ge(sM, t + 1)
                if t >= 3:
                    s.wait_ge(sS2, t - 2)    # Lb[t%3] free again
                s.activation(Lb[t % 3][:], xsl[t % NSLOT_][:], Act.Sign,
                             bias=m[:, c(t)], scale=-1.0,
                             accum_out=sg[:, c(t)]).then_inc(sL, 1)

            def p5a(t):
                s.wait_ge(sR, t + 1)
                s.activation(xsl[t % NSLOT_][:, P5A0:D],
                             xsl[t % NSLOT_][:, P5A0:D],
                             Act.Copy, bias=0.0,
                             scale=rc[:, c(t)]).then_inc(sP5a, 1)

            def store_act(t):
                s.wait_ge(sP5d, t + 1)
                s.wait_ge(sP5p, t + 1)
                c0 = SPL_ + SSP_ if t < 10 else 2731
                s.dma_start(
                    out_dr[t * PT:(t + 1) * PT, c0:D],
                    xsl[t % NSLOT_][:, c0:D]
                ).then_inc(dSact, 16)

            for t in range(2):
                s.dma_start(
                    xsl[t][:, 1792:2560], x_in[t * PT:(t + 1) * PT, 1792:2560]
                ).then_inc(dLact, 16)
            for t in range(NT):
                if t < 2:
                    s.dma_start(
                        xsl[t + 2][:, 1792:2560],
                        x_in[(t + 2) * PT:(t + 3) * PT, 1792:2560]
                    ).then_inc(dLact, 16)
                if t >= 6:
                    p5a(t - 6)
                    store_act(t - 6)
                    tn = t - 6 + NSLOT_
                    if tn < NT:
                        s.dma_start(
                            xsl[tn % NSLOT_][:, LSP_:D],
                            x_in[tn * PT:(tn + 1) * PT, LSP_:D]
                        ).then_inc(dLact, 16)
                sign_op(t)
            for t in range(NT - 6, NT):
                p5a(t)
                store_act(t)
            s.wait_ge(dSsp, 16 * NT)

        @block.gpsimd
        def _(gp):
            def pen_op(t):
                gp.wait_ge(sL, t + 1)
                gp.tensor_tensor(out=Lb[t % 3][:], in0=Lb[t % 3][:],
                                 in1=xsl[t % NSLOT_][:],
                                 op=Alu.mult).then_inc(sPen, 1)

            def tiny_op(t):
                gp.tensor_scalar(ta[:, c(t)], m[:, c(t)], 2.0, None,
                                 op0=Alu.subtract)
                gp.tensor_scalar(tb[:, c(t)], m[:, c(t)], 0.03125, None,
                                 op0=Alu.subtract)
                # dup <=> cnt >= 2 <=> sum(L) <= 4094
                gp.tensor_scalar(qq[:, c(t)], sg[:, c(t)], float(D - 2) + 0.5,
                                 None, op0=Alu.is_le)
                gp.drain()
                gp.tensor_scalar(qq[:, c(t)], qq[:, c(t)], 1e30, 1e30,
                                 op0=Alu.mult, op1=Alu.subtract)
                gp.drain()
                gp.tensor_scalar(qq[:, c(t)], qq[:, c(t)], tb[:, c(t)], None,
                                 op0=Alu.add)
                gp.wait_ge(sS2, t + 1)
                gp.tensor_scalar(tw[:, c(t)], s2[:, c(t)], ta[:, c(t)], None,
                                 op0=Alu.max)
                gp.drain()
                gp.tensor_scalar(tw[:, c(t)], tw[:, c(t)], tb[:, c(t)], None,
                                 op0=Alu.min)
                gp.drain()
                gp.tensor_scalar(tw[:, c(t)], tw[:, c(t)], qq[:, c(t)], None,
                                 op0=Alu.max)
                gp.drain().then_inc(sTw, 1)

            def p5p(t):
                gp.wait_ge(sR, t + 1)
                gp.tensor_scalar(xsl[t % NSLOT_][:, P5D_:P5A0],
                                 xsl[t % NSLOT_][:, P5D_:P5A0],
                                 rc[:, c(t)], None,
                                 op0=Alu.mult).then_inc(sP5p, 1)

            def store_pool(t):
                gp.wait_ge(sP5d, t + 1)
                gp.wait_ge(sP5a, t + 1)
                gp.wait_ge(sP5p, t + 1)
                c1 = SPL_ if t < 10 else 1365
                gp.dma_start(
                    out_dr[t * PT:(t + 1) * PT, 0:c1],
                    xsl[t % NSLOT_][:, 0:c1]
                ).then_inc(dSpool, 16)

            for t in range(4):
                gp.dma_start(
                    xsl[t][:, 2560:D], x_in[t * PT:(t + 1) * PT, 2560:D]
                ).then_inc(dLpool, 16)
            for t in range(NT):
                if t >= 6:
                    p5p(t - 6)
                    store_pool(t - 6)
                if t >= 1:
                    pen_op(t - 1)
                if t + 4 < NSLOT_:
                    gp.dma_start(
                        xsl[t + 4][:, LSP_:D],
                        x_in[(t + 4) * PT:(t + 5) * PT, LSP_:D]
                    ).then_inc(dLp2, 16)
                if t >= 3:
                    tiny_op(t - 3)
            pen_op(NT - 1)
            tiny_op(NT - 3)
            tiny_op(NT - 2)
            tiny_op(NT - 1)
            for t in range(NT - 6, NT):
                p5p(t)
                store_pool(t)

    return nc


def kernel(X: np.ndarray) -> np.ndarray:
    assert X.shape == (B, S, D) and X.dtype == np.float32
    if "nc" not in _cached:
        _cached["nc"] = _build()
    nc = _cached["nc"]
    in_maps = [{"x": np.ascontiguousarray(X[c])} for c in range(NCORES)]
    res = run_bass_kernel_spmd(nc, in_maps, core_ids=list(range(NCORES)))
    out = np.stack([res.results[c]["out"] for c in range(NCORES)], axis=0)
    return out


# revision 8
# speedup vs baseline: 1.0151x; 1.0046x over previous
"""EntmaxBisect (alpha=1.5, N_ITER=50, dim=-1) Trainium2 Bass kernel.

Math (host-validated, 0 active-set mismatches vs the f32 jax reference on
the seed-0 input): with p = 1/4095, u^p in [0.9958, 1.0002] for every
positive f32 u, so the normalized output is 1/k on the k elements at or
above the bisection threshold and 0 elsewhere (max elementwise deviation
2.3e-3, aggregate 1.6e-3 vs the 2e-2 gate).  The 50-step bisection
threshold collapses to twot = clamp(s2, m-2, m-1/32) in x units, where
s2 = max over non-max elements; duplicated max (cnt >= 2) converges to
t_max, i.e. twot = m - 1/32 exactly.

Engine-legal pass structure (BIR-verifier constraints: Pool = elementwise
tensor_scalar / tensor_tensor only, no accum, no stt; accums live on DVE
and ACT; hw-probed Sign(0) == 0):
  P1   m    = max-accum(x)                 tensor_scalar      [DVE]
  L    L    = Sign(m - x)  (1 below max, 0 at copies)
       cnt  = 4096 - add-accum(L)          activation         [ACT]
  PEN  pen  = L * x   (in-place into L; 0 at copies, s2 > 0)
                                           tensor_tensor      [Pool]
  S2   s2   = max-accum(pen)  (max select is exact)           [DVE]
  P45  mask = (x >= twot), k = add-accum  (in-place, full)    [DVE]
  R    r    = reciprocal(k)                                   [DVE tiny]
  P5   out  = mask * r  (in-place, column-split)       [DVE+Pool+ACT]
Tiny twot chain on Pool (ts-with-column-scalar forms only; dup override
arithmetic: tw = max(tw, dup*1e30 - 1e30 + (m-1/32))).
DMA: loads SP/ACT, stores SP/ACT/Pool (independent queues in the cost
model; DVE cannot DMA).  Software pipeline lags: L/pen@t, s2@t-1,
tiny@t-1, p45@t-2, r@t-3, p5/stores@t-4, over an NSLOT=8 slot ring.
Sharding: batch dim across the 8 cores, tile = 128 rows x 4096.
"""
import numpy as np
import concourse.bass as bass
import concourse.mybir as mybir
from concourse.bass_utils import run_bass_kernel_spmd
from contextlib import ExitStack

f32 = mybir.dt.float32
Alu = mybir.AluOpType
Act = mybir.ActivationFunctionType

B, S, D = 8, 2048, 4096
NCORES = 8
R = B * S // NCORES            # rows per core (2048)
PT = 128                       # partitions per tile
NT = R // PT                   # 16 tiles per core
LT = NT - 1                    # last tile: P5 fully on DVE at the tail

NSLOT = 8
LSP = 3328                     # load cols on SP queue; rest on ACT
SPL = 1408                     # store cols on Pool queue
SSP = 1920                     # store cols on SP queue (after Pool's)
# ACT stores the remainder [SPL+SSP : D) == its own load range; the
# queue-decoupling requires SPL + SSP == LSP (loads on one queue must
# not overlap stores pending on another queue's semaphore)
P5D = 1792                     # P5 cols on DVE
P5P = 1152                     # P5 cols on Pool; ACT takes the rest

_cached = {}


def _build(**over):
    g = dict(NSLOT=NSLOT, LSP=LSP, SPL=SPL, SSP=SSP, P5D=P5D, P5P=P5P)
    g.update(over)
    NSLOT_, LSP_, SPL_, SSP_, P5D_, P5P_ = (g["NSLOT"], g["LSP"], g["SPL"],
                                            g["SSP"], g["P5D"], g["P5P"])
    nc = bass.Bass(detect_race_conditions=False)
    x_in = nc.dram_tensor("x", [R, D], f32, kind="ExternalInput")
    out_dr = nc.dram_tensor("out", [R, D], f32, kind="ExternalOutput")

    with ExitStack() as st:
        block = st.enter_context(nc.Block())

        def sem(name):
            return st.enter_context(nc.semaphore(name))

        dLsp, dLact, dLpool = sem("dLsp"), sem("dLact"), sem("dLpool")
        dLp2 = sem("dLp2")
        dSsp, dSact, dSpool = sem("dSsp"), sem("dSact"), sem("dSpool")
        sM, sL, sPen, sS2 = sem("sM"), sem("sL"), sem("sPen"), sem("sS2")
        sTw, sR = sem("sTw"), sem("sR")
        sP5d, sP5p, sP5a, sP5L = (sem("sP5d"), sem("sP5p"), sem("sP5a"),
                                  sem("sP5L"))

        def sb(name, shape, dt=f32):
            return st.enter_context(nc.sbuf_tensor(name, shape, dt))

        xsl = [sb(f"x{i}", [PT, D]) for i in range(NSLOT_)]
        Lb = [sb(f"L{i}", [PT, D]) for i in range(3)]
        junk = sb("junk", [PT, D])
        m = sb("m", [PT, NT])
        sg = sb("sg", [PT, NT])
        s2 = sb("s2", [PT, NT])
        ta = sb("ta", [PT, NT])
        tb = sb("tb", [PT, NT])
        tw = sb("tw", [PT, NT])
        qq = sb("qq", [PT, NT])
        kc = sb("kc", [PT, NT])
        rc = sb("rc", [PT, NT])

        def c(t):
            return slice(t, t + 1)

        P5A0 = P5D_ + P5P_      # ACT P5 range start

        @block.sync
        def _(sync):
            sync.dma_start(
                xsl[0][:, 0:1365], x_in[0:PT, 0:1365]).then_inc(dLsp, 16)
            for t in range(1, 4):
                sync.dma_start(
                    xsl[t][:, 0:1792], x_in[t * PT:(t + 1) * PT, 0:1792]
                ).then_inc(dLsp, 16)
            for t in range(4, NSLOT_):
                sync.dma_start(
                    xsl[t][:, 0:LSP_], x_in[t * PT:(t + 1) * PT, 0:LSP_]
                ).then_inc(dLsp, 16)
            for t in range(NT):
                sync.wait_ge(sP5d, t + 1)
                sync.wait_ge(sP5p, t + 1)
                sync.wait_ge(sP5a, t + 1)
                c0, c1 = (SPL_, SPL_ + SSP_) if t < 10 else (1365, 2731)
                sync.dma_start(
                    out_dr[t * PT:(t + 1) * PT, c0:c1],
                    xsl[t % NSLOT_][:, c0:c1]
                ).then_inc(dSsp, 16)
                tn = t + NSLOT_
                if tn < NT:
                    sync.wait_ge(dSpool, 16 * (t + 1))
                    sync.dma_start(
                        xsl[tn % NSLOT_][:, 0:LSP_],
                        x_in[tn * PT:(tn + 1) * PT, 0:LSP_]
                    ).then_inc(dLsp, 16)
            sync.wait_ge(dSact, 16 * NT)
            sync.wait_ge(dSpool, 16 * NT)
            sync.wait_ge(dSsp, 16 * NT)

        @block.vector
        def _(v):
            def p1(t):
                v.wait_ge(dLsp, 16 * (t + 1))
                if t < 4:
                    v.wait_ge(dLact, 16 * (t + 1))
                    v.wait_ge(dLpool, 16 * (t + 1))
                elif t < 8:
                    v.wait_ge(dLp2, 16 * (t - 3))
                else:
                    v.wait_ge(dLact, 16 * (t - 3))
                v.tensor_scalar(junk[:], xsl[t % NSLOT_][:], 0.0, None,
                                op0=Alu.bypass, op1=Alu.max,
                                accum_out=m[:, c(t)]).then_inc(sM, 1)

            def s2op(t):
                v.wait_ge(sPen, t + 1)
                v.tensor_scalar(junk[:], Lb[t % 3][:], 0.0, None,
                                op0=Alu.bypass, op1=Alu.max,
                                accum_out=s2[:, c(t)]).then_inc(sS2, 1)

            def p45(t):
                v.wait_ge(sTw, t + 1)
                v.tensor_scalar(xsl[t % NSLOT_][:], xsl[t % NSLOT_][:],
                                tw[:, c(t)], None,
                                op0=Alu.is_ge, op1=Alu.add,
                                accum_out=kc[:, c(t)])

            def recip(t):
                v.reciprocal(rc[:, c(t)], kc[:, c(t)]).then_inc(sR, 1)

            def p5d(t):
                v.tensor_scalar(xsl[t % NSLOT_][:, 0:P5D_],
                                xsl[t % NSLOT_][:, 0:P5D_], rc[:, c(t)], None,
                                op0=Alu.mult).then_inc(sP5d, 1)

            for t in range(NT):
                p1(t)
                if t >= 6:
                    p5d(t - 6)
                if t >= 5:
                    recip(t - 5)
                if t >= 2:
                    s2op(t - 2)
                if t >= 4:
                    p45(t - 4)
            s2op(NT - 2)
            p45(NT - 4)
            recip(NT - 5)
            p5d(NT - 6)
            s2op(NT - 1)
            p45(NT - 3)
            recip(NT - 4)
            p5d(NT - 5)
            p45(NT - 2)
            recip(NT - 3)
            p5d(NT - 4)
            p45(NT - 1)
            recip(NT - 2)
            p5d(NT - 3)
            recip(NT - 1)
            p5d(NT - 2)
            p5d(NT - 1)

        @block.scalar
        def _(s):
            def sign_op(t):
                s.wait_ge(sM, t + 1)
                if t >= 3:
                    s.wait_ge(sS2, t - 2)    # Lb[t%3] free again
                s.activation(Lb[t % 3][:], xsl[t % NSLOT_][:], Act.Sign,
                             bias=m[:, c(t)], scale=-1.0,
                             accum_out=sg[:, c(t)]).then_inc(sL, 1)

            def p5a(t):
                s.wait_ge(sR, t + 1)
                s.activation(xsl[t % NSLOT_][:, P5A0:D],
                             xsl[t % NSLOT_][:, P5A0:D],
                             Act.Copy, bias=0.0,
                             scale=rc[:, c(t)]).then_inc(sP5a, 1)

            def store_act(t):
                s.wait_ge(sP5d, t + 1)
                s.wait_ge(sP5p, t + 1)
                c0 = SPL_ + SSP_ if t < 10 else 2731
                s.dma_start(
                    out_dr[t * PT:(t + 1) * PT, c0:D],
                    xsl[t % NSLOT_][:, c0:D]
                ).then_inc(dSact, 16)

            s.dma_start(
                xsl[0][:, 1365:2730], x_in[0:PT, 1365:2730]).then_inc(dLact, 16)
            s.dma_start(
                xsl[1][:, 1792:2560], x_in[PT:2 * PT, 1792:2560]
            ).then_inc(dLact, 16)
            for t in range(NT):
                if t < 2:
                    s.dma_start(
                        xsl[t + 2][:, 1792:2560],
                        x_in[(t + 2) * PT:(t + 3) * PT, 1792:2560]
                    ).then_inc(dLact, 16)
                if t >= 6:
                    p5a(t - 6)
                    store_act(t - 6)
                    tn = t - 6 + NSLOT_
                    if tn < NT:
                        s.dma_start(
                            xsl[tn % NSLOT_][:, LSP_:D],
                            x_in[tn * PT:(tn + 1) * PT, LSP_:D]
                        ).then_inc(dLact, 16)
                sign_op(t)
            for t in range(NT - 6, NT):
                p5a(t)
                store_act(t)
            s.wait_ge(dSsp, 16 * NT)

        @block.gpsimd
        def _(gp):
            def pen_op(t):
                gp.wait_ge(sL, t + 1)
                gp.tensor_tensor(out=Lb[t % 3][:], in0=Lb[t % 3][:],
                                 in1=xsl[t % NSLOT_][:],
                                 op=Alu.mult).then_inc(sPen, 1)

            def tiny_op(t):
                gp.tensor_scalar(ta[:, c(t)], m[:, c(t)], 2.0, None,
                                 op0=Alu.subtract)
                gp.tensor_scalar(tb[:, c(t)], m[:, c(t)], 0.03125, None,
                                 op0=Alu.subtract)
                # dup <=> cnt >= 2 <=> sum(L) <= 4094
                gp.tensor_scalar(qq[:, c(t)], sg[:, c(t)], float(D - 2) + 0.5,
                                 None, op0=Alu.is_le)
                gp.drain()
                gp.tensor_scalar(qq[:, c(t)], qq[:, c(t)], 1e30, 1e30,
                                 op0=Alu.mult, op1=Alu.subtract)
                gp.drain()
                gp.tensor_scalar(qq[:, c(t)], qq[:, c(t)], tb[:, c(t)], None,
                                 op0=Alu.add)
                gp.wait_ge(sS2, t + 1)
                gp.tensor_scalar(tw[:, c(t)], s2[:, c(t)], ta[:, c(t)], None,
                                 op0=Alu.max)
                gp.drain()
                gp.tensor_scalar(tw[:, c(t)], tw[:, c(t)], tb[:, c(t)], None,
                                 op0=Alu.min)
                gp.drain()
                gp.tensor_scalar(tw[:, c(t)], tw[:, c(t)], qq[:, c(t)], None,
                                 op0=Alu.max)
                gp.drain().then_inc(sTw, 1)

            def p5p(t):
                gp.wait_ge(sR, t + 1)
                gp.tensor_scalar(xsl[t % NSLOT_][:, P5D_:P5A0],
                                 xsl[t % NSLOT_][:, P5D_:P5A0],
                                 rc[:, c(t)], None,
                                 op0=Alu.mult).then_inc(sP5p, 1)

            def store_pool(t):
                gp.wait_ge(sP5d, t + 1)
                gp.wait_ge(sP5a, t + 1)
                gp.wait_ge(sP5p, t + 1)
                c1 = SPL_ if t < 10 else 1365
                gp.dma_start(
                    out_dr[t * PT:(t + 1) * PT, 0:c1],
                    xsl[t % NSLOT_][:, 0:c1]
                ).then_inc(dSpool, 16)

            gp.dma_start(
                xsl[0][:, 2730:D], x_in[0:PT, 2730:D]).then_inc(dLpool, 16)
            for t in range(1, 4):
                gp.dma_start(
                    xsl[t][:, 2560:D], x_in[t * PT:(t + 1) * PT, 2560:D]
                ).then_inc(dLpool, 16)
            for t in range(NT):
                if t >= 6:
                    p5p(t - 6)
                    store_pool(t - 6)
                if t >= 1:
                    pen_op(t - 1)
                if t + 4 < NSLOT_:
                    gp.dma_start(
                        xsl[t + 4][:, LSP_:D],
                        x_in[(t + 4) * PT:(t + 5) * PT, LSP_:D]
                    ).then_inc(dLp2, 16)
                if t >= 3:
                    tiny_op(t - 3)
            pen_op(NT - 1)
            tiny_op(NT - 3)
            tiny_op(NT - 2)
            tiny_op(NT - 1)
            for t in range(NT - 6, NT):
                p5p(t)
                store_pool(t)

    return nc


def kernel(X: np.ndarray) -> np.ndarray:
    assert X.shape == (B, S, D) and X.dtype == np.float32
    if "nc" not in _cached:
        _cached["nc"] = _build()
    nc = _cached["nc"]
    in_maps = [{"x": np.ascontiguousarray(X[c])} for c in range(NCORES)]
    res = run_bass_kernel_spmd(nc, in_maps, core_ids=list(range(NCORES)))
    out = np.stack([res.results[c]["out"] for c in range(NCORES)], axis=0)
    return out
